# revision 1
# baseline (speedup 1.0000x reference)
"""Trainium2 Bass kernel: 2-layer LSTM decoder with embedding lookup.

Reference computation (per nn.Decoder):
    tgt_embed = emb[prev_tgt_tokens]                      # [B, T, D]
    for t in 0..T-1:
        x = tgt_embed[:, t]
        for l in 0..1:
            gates = x @ W_ih[l].T + b_ih[l] + h[l] @ W_hh[l].T + b_hh[l]
            i, f, g, o = split(gates, 4)
            c[l] = sigmoid(f) * c[l] + sigmoid(i) * tanh(g)
            h[l] = sigmoid(o) * tanh(c[l])
            x = h[l]
        out[:, t] = h[1]

Sharding: data-parallel over batch B=64 across 8 cores (8 rows each);
embedding + LSTM weights replicated; the sequential time loop runs
on-device per core, fully unrolled.

Kernel design (per core):
  - Embedding gather via indirect DMA (128 rows per call), PE-transposed
    into K-major layout.
  - Input projection x @ W_ih[0].T batched over all T steps as one big
    matmul, spilled to a DRAM scratch and staged back per step.
  - Recurrent loop: gates in [8(batch part), 2048(free)] layout; the
    h @ W_hh.T matmuls keep h^T as the (tiny) stationary operand and
    stream W^T as moving data, which is the fp32 throughput-optimal
    orientation. Layer-0 of step t is interleaved with layer-1 of step
    t-1 so the activation chains hide under PE work.
"""

import os

import numpy as np

import concourse.bass as bass
import concourse.mybir as mybir
import concourse.tile as tile
from concourse import bacc
from concourse.bass_utils import run_bass_kernel_spmd
from concourse.masks import make_identity

N_CORES = 8
B = 64
T = int(os.environ.get("BASS_LSTM_T", "128"))
D = 512
V = 32000
G = 4 * D            # 2048 gate dims per layer
BL = B // N_CORES    # 8 batch rows per core
KC = D // 128        # 4 contraction chunks of 128
NB = G // 512        # 4 PSUM banks of 512 per gate vector
MT = BL * T // 128   # M-tiles (128 token rows each) for the input matmul
REPS = int(os.environ.get("BASS_LSTM_REPS", "1"))  # timing-only: loop phase B
ABLATE = os.environ.get("BASS_ABLATE", "")  # "", "mmonly", "notrans" (sim experiments)
F32 = mybir.dt.float32
I32 = mybir.dt.int32
AFT = mybir.ActivationFunctionType

# Matmul compute dtype. float32r is the fast fp32 PE mode: 1 cycle/row for
# moving free dim >= 256 vs 4 cycles/row for plain fp32 (2 half-speed passes).
# The BIR verifier requires fp32r matmul operands to be produced by an
# instruction that rounds to fp32r, so operand tiles are declared fp32r and
# filled via converting DVE copies.
WDT = {
    "f32": F32,
    "f32r": mybir.dt.float32r,
}[os.environ.get("BASS_LSTM_MMDT", "f32r")]

# gate banks after host-side permutation: [f, i, g, o]
BANK_F, BANK_I, BANK_G, BANK_O = 0, 1, 2, 3
R1 = 32  # partition row where the layer-1 lane starts (32-aligned for PE)


def _nsl(n):
    return slice(n * 512, (n + 1) * 512)


def _build():
    nc = bacc.Bacc(
        "TRN2",
        target_bir_lowering=False,
        debug=False,
        enable_asserts=False,
        num_devices=N_CORES,
    )

    tok_d = nc.dram_tensor("tokens", [BL * T, 1], I32, kind="ExternalInput")
    emb_d = nc.dram_tensor("emb", [V, D], F32, kind="ExternalInput")
    wih0_d = nc.dram_tensor("wih0t", [D, G], F32, kind="ExternalInput")
    whh0_d = nc.dram_tensor("whh0t", [D, G], F32, kind="ExternalInput")
    wih1_d = nc.dram_tensor("wih1t", [D, G], F32, kind="ExternalInput")
    whh1_d = nc.dram_tensor("whh1t", [D, G], F32, kind="ExternalInput")
    bias_d = nc.dram_tensor("bias", [2, 128, G], F32, kind="ExternalInput")
    ht_d = nc.dram_tensor("ht_init", [2, 128, KC * BL], F32, kind="ExternalInput")
    c_d = nc.dram_tensor("c_init", [2, BL, D], F32, kind="ExternalInput")
    out_d = nc.dram_tensor("out", [BL, T, D], F32, kind="ExternalOutput")

    with tile.TileContext(nc) as tc:
        _body(
            tc,
            tok=tok_d.ap(),
            emb=emb_d.ap(),
            w=[wih0_d.ap(), whh0_d.ap(), wih1_d.ap(), whh1_d.ap()],
            bias=bias_d.ap(),
            ht0=ht_d.ap(),
            c0=c_d.ap(),
            out=out_d.ap(),
        )
    nc.compile()
    return nc


def _body(tc, tok, emb, w, bias, ht0, c0, out):
    nc = tc.nc
    with (
        tc.tile_pool(name="wpool", bufs=1) as wp,
        tc.tile_pool(name="dram", bufs=1, space="DRAM") as dr,
        tc.tile_pool(name="state", bufs=1) as st,
        tc.tile_pool(name="work", bufs=2) as wk,
        tc.tile_pool(name="pspool", bufs=4, space="PSUM") as pp,
    ):
        # ---- persistent tiles -------------------------------------------
        id_sb = wp.tile([128, 128], F32)
        make_identity(nc, id_sb[:])

        whh0_sb = wp.tile([128, KC * G], WDT)
        wih1_sb = wp.tile([128, KC * G], WDT)
        whh1_sb = wp.tile([128, KC * G], WDT)

        def load_w(dst, src_ap, ci):
            # DMA one K-chunk to an fp32 stage, then convert-copy into the
            # fp32r-typed resident tile.
            for c in range(KC):
                stg = wk.tile(
                    [128, G], F32, tag=f"g{(ci + c) % 2}", bufs=1, name="wstage"
                )
                nc.sync.dma_start(
                    out=stg[:],
                    in_=src_ap.rearrange("(c p) n -> p c n", p=128)[:, c, :],
                )
                nc.vector.tensor_copy(out=dst[:, c * G : (c + 1) * G], in_=stg[:])

        load_w(whh0_sb, w[1], 0)
        load_w(wih1_sb, w[2], 1)
        load_w(whh1_sb, w[3], 0)

        bias1_sb = wp.tile([BL, G], F32)
        nc.sync.dma_start(out=bias1_sb[:], in_=bias[1, :BL, :])

        bias1_sb = wp.tile([BL, G], F32)
        nc.sync.dma_start(out=bias1_sb[:], in_=bias[1, :BL, :])

        # input projection for all steps, spilled to DRAM scratch
        gx_dram = dr.tile([MT * 128, G], F32)

        # LSTM state + persistent chain tiles. Layer 0 (step t) lives on
        # partitions 0:8, layer 1 (step t-1) on partitions 32:40 ("lanes"),
        # so one elementwise op handles both layers (DVE/ACT cost scales
        # with the free dim only). Rows between the lanes hold junk that is
        # memset once and never published.
        NR = R1 + BL  # 40 partition rows
        hT = [None, None]  # [128, KC*BL], h^T packed
        for l in range(2):
            hstg = wk.tile([128, KC * BL], F32, tag="h", name="hstg")
            nc.sync.dma_start(out=hstg[:], in_=ht0[l])
            t0 = st.tile([128, KC * BL], WDT, tag=f"ht{l}", bufs=2)
            nc.vector.tensor_copy(out=t0[:], in_=hstg[:])
            hT[l] = t0

        cst = st.tile([NR, D], F32)
        gt = st.tile([NR, G], F32)
        fct = st.tile([NR, D], F32)
        mt_ = st.tile([NR, D], F32)
        tch = st.tile([NR, D], F32)
        hst = st.tile([NR, D], F32)
        for tile_ in (cst, gt, fct, mt_, tch, hst):
            nc.vector.memset(tile_[:], 0.0)
        nc.sync.dma_start(out=cst[:BL, :], in_=c0[0])
        nc.sync.dma_start(out=cst[R1 : R1 + BL, :], in_=c0[1])

        # ---- phase A: gather + transpose + batched input projection ----
        with tc.tile_pool(name="ph0", bufs=1) as p0:
            wih0_sb = p0.tile([128, KC * G], WDT)
            load_w(wih0_sb, w[0], 1)
            bias0_bc = p0.tile([128, G], F32)
            nc.sync.dma_start(out=bias0_bc[:], in_=bias[0])

            for m in range(MT):
                idx_m = p0.tile([128, 1], I32, tag="idx", bufs=2)
                nc.sync.dma_start(out=idx_m[:], in_=tok[m * 128 : (m + 1) * 128, :])
                emb_m = p0.tile([128, D], F32, tag="embrows", bufs=1)
                nc.gpsimd.indirect_dma_start(
                    out=emb_m[:],
                    out_offset=None,
                    in_=emb,
                    in_offset=bass.IndirectOffsetOnAxis(ap=idx_m[:, :1], axis=0),
                )
                # transpose [tb, d] -> [d, tb] per 128-chunk of d
                pst = pp.tile([128, D], F32, tag="ps")
                for c in range(KC):
                    nc.tensor.transpose(
                        out=pst[:, c * 128 : (c + 1) * 128],
                        in_=emb_m[:, c * 128 : (c + 1) * 128],
                        identity=id_sb[:],
                    )
                embT_m = p0.tile([128, D], WDT, tag="embT", bufs=1)
                for c in range(KC):
                    nc.vector.tensor_copy(
                        out=embT_m[:, c * 128 : (c + 1) * 128],
                        in_=pst[:, c * 128 : (c + 1) * 128],
                    )
                # batched input matmul for this M-tile (per-bank psum slots)
                gxs = wk.tile([128, G], F32, tag="g0", bufs=1, name="gxs")
                for n in range(NB):
                    psm = pp.tile([128, 512], F32, tag="ps", name="psm")
                    for c in range(KC):
                        nc.tensor.matmul(
                            out=psm[:, :],
                            lhsT=embT_m[:, c * 128 : (c + 1) * 128],
                            rhs=wih0_sb[:, c * G + n * 512 : c * G + (n + 1) * 512],
                            start=(c == 0),
                            stop=(c == KC - 1),
                        )
                    nc.vector.tensor_add(
                        out=gxs[:, _nsl(n)], in0=psm[:, :], in1=bias0_bc[:, _nsl(n)]
                    )
                nc.sync.dma_start(
                    out=gx_dram[m * 128 : (m + 1) * 128, :], in_=gxs[:]
                )

        # ---- phase B: recurrent loop ------------------------------------
        # Iteration t emits layer-0 matmuls for step t and layer-1 matmuls
        # for step t-1 into shared per-bank PSUM tiles (lanes 0:8 / 32:40),
        # then one stacked activation chain for both. All activations are
        # Sigmoid (tanh(x) = 2*sigmoid(2x) - 1) so the ACT engine never
        # reloads its function table (1.3us per switch).
        FI, GSL, OSL = slice(0, 1024), _nsl(BANK_G), _nsl(BANK_O)

        def zero_psum():
            # One-time scrub so stacked chain ops can read the junk rows
            # between the lanes without tripping finite-checks.
            for _ in range(4):
                z = pp.tile([128, 1024], F32, tag="ps", name="pz")
                nc.vector.memset(z[:], 0.0)

        zero_psum()

        def mm_group(pb, col0, stat, w_sb, n, start, stop, rows=slice(0, BL)):
            for c in range(KC):
                nc.tensor.matmul(
                    out=pb[rows, col0 : col0 + 512],
                    lhsT=stat[:, c * BL : (c + 1) * BL],
                    rhs=w_sb[:, c * G + n * 512 : c * G + (n + 1) * 512],
                    start=start and c == 0,
                    stop=stop and c == KC - 1,
                )

        for rep in range(REPS):
          for t in range(T + 1):
            last = t == T
            first = t == 0
            gxt = None
            if not last:
                gxt = wk.tile([BL, G], F32, tag="gxt", bufs=3)
                nc.sync.dma_start(
                    out=gxt[:], in_=gx_dram[t * BL : (t + 1) * BL, :]
                )

            # matmuls: separate per-lane psum tiles so each bank+lane is one
            # uninterrupted accumulation group (keeps the PE ramp warm)
            pb_fi0 = pb_go0 = pb_fi1 = pb_go1 = None
            if not last:
                pb_fi0 = pp.tile([128, 1024], F32, tag="ps", name="pb_fi0")
                pb_go0 = pp.tile([128, 1024], F32, tag="ps", name="pb_go0")
            if not first:
                pb_fi1 = pp.tile([128, 1024], F32, tag="ps", name="pb_fi1")
                pb_go1 = pp.tile([128, 1024], F32, tag="ps", name="pb_go1")
            for bi, (p0_, p1_, col0) in enumerate(
                [
                    (pb_fi0, pb_fi1, 0),
                    (pb_fi0, pb_fi1, 512),
                    (pb_go0, pb_go1, 0),
                    (pb_go0, pb_go1, 512),
                ]
            ):
                if not last:
                    mm_group(p0_, col0, hT[0], whh0_sb, bi, True, True, slice(0, BL))
                if not first:
                    mm_group(p1_, col0, hT[0], wih1_sb, bi, True, False, slice(0, BL))
                    mm_group(p1_, col0, hT[1], whh1_sb, bi, False, True, slice(0, BL))

            # per-lane gate adds (lane inputs differ), stacked everything else
            L0, L1 = slice(0, BL), slice(R1, R1 + BL)
            if not last:
                nc.vector.tensor_add(
                    out=gt[L0, FI], in0=pb_fi0[:BL, :], in1=gxt[:, FI]
                )
            if not first:
                nc.vector.tensor_add(
                    out=gt[L1, FI], in0=pb_fi1[:BL, :], in1=bias1_sb[:, FI]
                )
            nc.scalar.activation(out=gt[:, FI], in_=gt[:, FI], func=AFT.Sigmoid)
            nc.vector.tensor_mul(out=fct[:], in0=gt[:, _nsl(BANK_F)], in1=cst[:])
            # fi_diff = f*c - sig_i  (the "- i" term of i*(2s_g - 1))
            nc.vector.tensor_sub(out=fct[:], in0=fct[:], in1=gt[:, _nsl(BANK_I)])
            if not last:
                nc.vector.tensor_add(
                    out=gt[L0, GSL], in0=pb_go0[:BL, 0:512], in1=gxt[:, GSL]
                )
            if not first:
                nc.vector.tensor_add(
                    out=gt[L1, GSL], in0=pb_go1[:BL, 0:512], in1=bias1_sb[:, GSL]
                )
            nc.scalar.activation(
                out=gt[:, GSL], in_=gt[:, GSL], func=AFT.Sigmoid, scale=2.0
            )
            # m = 2 * sig_g * sig_i ; c = fi_diff + m
            nc.vector.scalar_tensor_tensor(
                out=mt_[:], in0=gt[:, GSL], scalar=2.0, in1=gt[:, _nsl(BANK_I)],
                op0=mybir.AluOpType.mult, op1=mybir.AluOpType.mult,
            )
            crows = slice(0, BL) if first else (slice(R1, R1 + BL) if last
                                                else slice(0, NR))
            nc.vector.tensor_add(out=cst[crows, :], in0=fct[crows, :],
                                 in1=mt_[crows, :])
            # tanh(c) = 2*sigmoid(2c) - 1
            nc.scalar.activation(out=tch[:], in_=cst[:], func=AFT.Sigmoid, scale=2.0)
            nc.vector.tensor_scalar(
                out=tch[:], in0=tch[:], scalar1=2.0, scalar2=-1.0,
                op0=mybir.AluOpType.mult, op1=mybir.AluOpType.add,
            )
            if not last:
                nc.vector.tensor_add(
                    out=gt[L0, OSL], in0=pb_go0[:BL, 512:1024], in1=gxt[:, OSL]
                )
            if not first:
                nc.vector.tensor_add(
                    out=gt[L1, OSL], in0=pb_go1[:BL, 512:1024], in1=bias1_sb[:, OSL]
                )
            nc.scalar.activation(out=gt[:, OSL], in_=gt[:, OSL], func=AFT.Sigmoid)
            nc.vector.tensor_mul(out=hst[:], in0=gt[:, OSL], in1=tch[:])
            if not first:
                nc.sync.dma_start(out=out[:, t - 1, :], in_=hst[R1 : R1 + BL, :])

            # h -> h^T transposes into the consumed G-regions + f32r copies
            if not last:
                for c in range(KC):
                    nc.tensor.transpose(
                        out=pb_go0[:, c * BL : (c + 1) * BL],
                        in_=hst[:BL, c * 128 : (c + 1) * 128],
                        identity=id_sb[:BL, :BL],
                    )
                hT0n = st.tile([128, KC * BL], WDT, tag="ht0", bufs=2, name="hT0n")
                nc.vector.tensor_copy(out=hT0n[:], in_=pb_go0[:, 0 : KC * BL])
                hT[0] = hT0n
                if not first:
                    for c in range(KC):
                        nc.tensor.transpose(
                            out=pb_go1[:, c * BL : (c + 1) * BL],
                            in_=hst[R1 : R1 + BL, c * 128 : (c + 1) * 128],
                            identity=id_sb[R1 : R1 + BL, R1 : R1 + BL],
                        )
                    hT1n = st.tile(
                        [128, KC * BL], WDT, tag="ht1", bufs=2, name="hT1n"
                    )
                    nc.vector.tensor_copy(out=hT1n[:], in_=pb_go1[:, 0 : KC * BL])
                    hT[1] = hT1n


_NC_CACHE = {}


def _get_nc():
    if "nc" not in _NC_CACHE:
        _NC_CACHE["nc"] = _build()
    return _NC_CACHE["nc"]


def _make_in_maps(inputs):
    tokens = np.asarray(inputs["prev_tgt_tokens"])[:, :T].astype(np.int32)  # [B, T]
    emb = np.ascontiguousarray(np.asarray(inputs["emb"], dtype=np.float32))
    W_ih = np.asarray(inputs["W_ih"], dtype=np.float32)
    W_hh = np.asarray(inputs["W_hh"], dtype=np.float32)
    b_ih = np.asarray(inputs["b_ih"], dtype=np.float32)
    b_hh = np.asarray(inputs["b_hh"], dtype=np.float32)
    hiddens = np.asarray(inputs["hiddens"], dtype=np.float32)
    cells = np.asarray(inputs["cells"], dtype=np.float32)

    def permute_gates(a, axis):
        # PyTorch gate order [i, f, g, o] -> kernel bank order [f, i, g, o]
        blocks = np.split(a, 4, axis=axis)
        return np.concatenate([blocks[1], blocks[0], blocks[2], blocks[3]], axis=axis)

    wih0t = np.ascontiguousarray(permute_gates(W_ih[0].T, 1))  # [D, G]
    whh0t = np.ascontiguousarray(permute_gates(W_hh[0].T, 1))
    wih1t = np.ascontiguousarray(permute_gates(W_ih[1].T, 1))
    whh1t = np.ascontiguousarray(permute_gates(W_hh[1].T, 1))
    bias = np.ascontiguousarray(
        np.broadcast_to(
            permute_gates(b_ih + b_hh, 1)[:, None, :], (2, 128, G)
        ).astype(np.float32)
    )

    in_maps = []
    for core in range(N_CORES):
        sl = slice(core * BL, (core + 1) * BL)
        tok_tm = np.ascontiguousarray(tokens[sl].T.reshape(BL * T, 1))  # t-major
        ht = np.empty((2, 128, KC * BL), dtype=np.float32)
        for l in range(2):
            # [BL, D] -> h^T [D, BL] -> [KC, 128, BL] -> [128, KC, BL]
            htl = hiddens[l, sl].T.reshape(KC, 128, BL).transpose(1, 0, 2)
            ht[l] = htl.reshape(128, KC * BL)
        cin = np.ascontiguousarray(cells[:, sl, :])
        in_maps.append(
            {
                "tokens": tok_tm,
                "emb": emb,
                "wih0t": wih0t,
                "whh0t": whh0t,
                "wih1t": wih1t,
                "whh1t": whh1t,
                "bias": bias,
                "ht_init": np.ascontiguousarray(ht),
                "c_init": cin,
            }
        )
    return in_maps


def run(inputs, trace=False, **kwargs):
    """Build (cached), run on 8 cores, return (full_output, BassKernelResults)."""
    nc = _get_nc()
    in_maps = _make_in_maps(inputs)
    res = run_bass_kernel_spmd(
        nc, in_maps, core_ids=list(range(N_CORES)), trace=trace, **kwargs
    )
    out = np.concatenate([r["out"] for r in res.results], axis=0)  # [B, T, D]
    return out, res


def kernel(**inputs) -> np.ndarray:
    out, _ = run(inputs, trace=False)
    return out



# revision 11
# speedup vs baseline: 1.5590x; 1.5590x over previous
"""Trainium2 Bass kernel: 2-layer LSTM decoder with embedding lookup.

Reference computation (per nn.Decoder):
    tgt_embed = emb[prev_tgt_tokens]                      # [B, T, D]
    for t in 0..T-1:
        x = tgt_embed[:, t]
        for l in 0..1:
            gates = x @ W_ih[l].T + b_ih[l] + h[l] @ W_hh[l].T + b_hh[l]
            i, f, g, o = split(gates, 4)
            c[l] = sigmoid(f) * c[l] + sigmoid(i) * tanh(g)
            h[l] = sigmoid(o) * tanh(c[l])
            x = h[l]
        out[:, t] = h[1]

Sharding: data-parallel over batch B=64 across 8 cores (8 rows each);
embedding + LSTM weights replicated; the sequential time loop runs
on-device per core, fully unrolled.

Kernel design (per core):
  - All matmul operands (emb rows, weights, h) are bf16, converted
    host-side so no on-device staging/convert copies are needed and the
    HBM weight traffic is halved. PSUM accumulation and the gate/state
    chain stay fp32.
  - Embedding gather via indirect DMA (128 rows per call), PE-transposed
    into K-major layout.
  - Input projection x @ W_ih[0].T batched over all T steps as one big
    matmul; the result stays resident in SBUF (bf16) instead of a DRAM
    round trip, and is staged per step with a small SBUF->SBUF DMA.
  - Recurrent loop: iteration t runs layer-0 step t and layer-1 step t-1.
    Each lane has its own PSUM tile and its own (unstacked) activation
    chain so the lane-0 recurrence - which gates the next iteration's
    matmuls - completes while the PE is still streaming lane-1 weights.
    The lane-1 h-transpose is deferred one further iteration so it never
    head-of-line blocks the PE queue.
  - All activations are Sigmoid (tanh(x) = 2*sigmoid(2x)-1) so the ACT
    engine never reloads its function table.
"""

import os

import numpy as np

import concourse.bass as bass
import concourse.mybir as mybir
import concourse.tile as tile
from concourse import bacc
from concourse.bass_utils import run_bass_kernel_spmd
from concourse.masks import make_identity

N_CORES = 8
B = 64
T = int(os.environ.get("BASS_LSTM_T", "128"))
D = 512
V = 32000
G = 4 * D            # 2048 gate dims per layer
BL = B // N_CORES    # 8 batch rows per core
KC = D // 128        # 4 contraction chunks of 128
NB = G // 512        # 4 PSUM banks of 512 per gate vector
MT = BL * T // 128   # M-tiles (128 token rows each) for the input matmul
TPM = 128 // BL      # time steps per M-tile (16)
REPS = int(os.environ.get("BASS_LSTM_REPS", "1"))  # timing-only: loop phase B
F32 = mybir.dt.float32
BF16 = mybir.dt.bfloat16
I32 = mybir.dt.int32
AFT = mybir.ActivationFunctionType

# gate banks after host-side permutation: [f, i, g, o]
BANK_F, BANK_I, BANK_G, BANK_O = 0, 1, 2, 3
FI, GSL, OSL = slice(0, 1024), slice(1024, 1536), slice(1536, 2048)


def _nsl(n):
    return slice(n * 512, (n + 1) * 512)


def _build():
    nc = bacc.Bacc(
        "TRN2",
        target_bir_lowering=False,
        debug=False,
        enable_asserts=False,
        num_devices=N_CORES,
    )

    tok_d = nc.dram_tensor("tokens", [BL * T, 1], I32, kind="ExternalInput")
    emb_d = nc.dram_tensor("emb", [V, D], BF16, kind="ExternalInput")
    wih0_d = nc.dram_tensor("wih0t", [D, G], BF16, kind="ExternalInput")
    whh0_d = nc.dram_tensor("whh0t", [D, G], BF16, kind="ExternalInput")
    wih1_d = nc.dram_tensor("wih1t", [D, G], BF16, kind="ExternalInput")
    whh1_d = nc.dram_tensor("whh1t", [D, G], BF16, kind="ExternalInput")
    bias0_d = nc.dram_tensor("bias0", [128, G], BF16, kind="ExternalInput")
    bias1_d = nc.dram_tensor("bias1", [BL, G], F32, kind="ExternalInput")
    ht_d = nc.dram_tensor("ht_init", [2, 128, KC * BL], BF16, kind="ExternalInput")
    c_d = nc.dram_tensor("c_init", [2, BL, D], F32, kind="ExternalInput")
    out_d = nc.dram_tensor("out", [BL, T, D], F32, kind="ExternalOutput")

    with tile.TileContext(nc) as tc:
        _body(
            tc,
            tok=tok_d.ap(),
            emb=emb_d.ap(),
            w=[wih0_d.ap(), whh0_d.ap(), wih1_d.ap(), whh1_d.ap()],
            bias0=bias0_d.ap(),
            bias1=bias1_d.ap(),
            ht0=ht_d.ap(),
            c0=c_d.ap(),
            out=out_d.ap(),
        )
    nc.compile()
    return nc


def _body(tc, tok, emb, w, bias0, bias1, ht0, c0, out):
    nc = tc.nc
    with (
        tc.tile_pool(name="wpool", bufs=1) as wp,
        tc.tile_pool(name="state", bufs=1) as st,
        tc.tile_pool(name="work", bufs=2) as wk,
        tc.tile_pool(name="pspool", bufs=1, space="PSUM") as pp,
    ):
        # ---- persistent tiles -------------------------------------------
        id_sb = wp.tile([128, 128], F32)
        make_identity(nc, id_sb[:])
        id_bf = wp.tile([128, 128], BF16)
        make_identity(nc, id_bf[:])

        whh0_sb = wp.tile([128, KC * G], BF16)
        wih1_sb = wp.tile([128, KC * G], BF16)
        whh1_sb = wp.tile([128, KC * G], BF16)

        def load_w(dst, src_ap):
            # one DMA: [D, G] viewed as [128, KC, G] chunk-major
            nc.sync.dma_start(
                out=dst[:].rearrange("p (c n) -> p c n", c=KC),
                in_=src_ap.rearrange("(c p) n -> p c n", p=128),
            )

        load_w(whh0_sb, w[1])
        load_w(wih1_sb, w[2])
        load_w(whh1_sb, w[3])

        bias1_sb = wp.tile([BL, G], F32)
        nc.sync.dma_start(out=bias1_sb[:], in_=bias1)

        # input projection for all steps, resident in SBUF (bf16)
        gx_sb = wp.tile([128, MT * G], BF16)

        # per-lane state tiles (both lanes at base partition 0)
        hT = [None, None]  # [128, KC*BL], h^T packed, bf16
        for l in range(2):
            t0 = st.tile([128, KC * BL], BF16, tag=f"ht{l}", bufs=2)
            nc.sync.dma_start(out=t0[:], in_=ht0[l])
            hT[l] = t0

        cst, gt, fct, mt_, tch = [], [], [], [], []
        hst = [None, None]
        for l in range(2):
            cst.append(st.tile([BL, D], F32, name=f"cst{l}"))
            gt.append(st.tile([BL, G], F32, name=f"gt{l}"))
            fct.append(st.tile([BL, D], F32, name=f"fct{l}"))
            mt_.append(st.tile([BL, D], F32, name=f"mt{l}"))
            tch.append(st.tile([BL, D], F32, name=f"tch{l}"))
            hst[l] = st.tile([BL, D], F32, tag=f"hst{l}", bufs=2,
                             name=f"hst{l}")
            nc.sync.dma_start(out=cst[l][:], in_=c0[l])

        # persistent per-lane PSUM tiles: 2 x [128, 2048] fp32 = all 8 banks
        pbl0 = pp.tile([128, G], F32, tag="pbl0", name="pbl0")
        pbl1 = pp.tile([128, G], F32, tag="pbl1", name="pbl1")

        # ---- phase A: gather + transpose + batched input projection ----
        with tc.tile_pool(name="ph0", bufs=1) as p0:
            wih0_sb = p0.tile([128, KC * G], BF16)
            load_w(wih0_sb, w[0])
            bias0_bc = p0.tile([128, G], BF16)
            nc.sync.dma_start(out=bias0_bc[:], in_=bias0)

            for m in range(MT):
                idx_m = p0.tile([128, 1], I32, tag="idx", bufs=2)
                nc.sync.dma_start(out=idx_m[:], in_=tok[m * 128 : (m + 1) * 128, :])
                emb_m = p0.tile([128, D], BF16, tag="embrows", bufs=1)
                nc.gpsimd.indirect_dma_start(
                    out=emb_m[:],
                    out_offset=None,
                    in_=emb,
                    in_offset=bass.IndirectOffsetOnAxis(ap=idx_m[:, :1], axis=0),
                )
                # transpose [tb, d] -> [d, tb] per 128-chunk of d
                pbl0_bf = pbl0[:].bitcast(BF16)
                for c in range(KC):
                    nc.tensor.transpose(
                        out=pbl0_bf[:, c * 128 : (c + 1) * 128],
                        in_=emb_m[:, c * 128 : (c + 1) * 128],
                        identity=id_bf[:],
                    )
                embT_m = p0.tile([128, D], BF16, tag="embT", bufs=1)
                nc.vector.tensor_copy(out=embT_m[:], in_=pbl0_bf[:, 0:D])
                # batched input matmul for this M-tile (per-bank psum slots)
                for n in range(NB):
                    for c in range(KC):
                        nc.tensor.matmul(
                            out=pbl1[:, _nsl(n)],
                            lhsT=embT_m[:, c * 128 : (c + 1) * 128],
                            rhs=wih0_sb[:, c * G + n * 512 : c * G + (n + 1) * 512],
                            start=(c == 0),
                            stop=(c == KC - 1),
                        )
                    nc.vector.tensor_add(
                        out=gx_sb[:, m * G + n * 512 : m * G + (n + 1) * 512],
                        in0=pbl1[:, _nsl(n)],
                        in1=bias0_bc[:, _nsl(n)],
                    )

        # ---- phase B: recurrent loop ------------------------------------
        # Iteration t: lane 0 = layer-0 step t, lane 1 = layer-1 step t-1.
        # Lane-1's h transpose for step t-2 runs at the top of iteration t.

        def mm_group(pb, col0, stat, w_sb, n, start, stop):
            for c in range(KC):
                nc.tensor.matmul(
                    out=pb[0:BL, col0 : col0 + 512],
                    lhsT=stat[:, c * BL : (c + 1) * BL],
                    rhs=w_sb[:, c * G + n * 512 : c * G + (n + 1) * 512],
                    start=start and c == 0,
                    stop=stop and c == KC - 1,
                )

        def transpose_h(pb, src):
            for c in range(KC):
                nc.tensor.transpose(
                    out=pb[:, c * BL : (c + 1) * BL],
                    in_=src[:BL, c * 128 : (c + 1) * 128],
                    identity=id_sb[:BL, :BL],
                )

        def chain(l):
            # gate adds are emitted by the caller; this emits the per-lane
            # activation chain.
            eng = nc.vector
            nc.scalar.activation(out=gt[l][:, FI], in_=gt[l][:, FI],
                                 func=AFT.Sigmoid)
            nc.scalar.activation(out=gt[l][:, GSL], in_=gt[l][:, GSL],
                                 func=AFT.Sigmoid, scale=2.0)
            nc.scalar.activation(out=gt[l][:, OSL], in_=gt[l][:, OSL],
                                 func=AFT.Sigmoid)
            eng.tensor_mul(out=fct[l][:], in0=gt[l][:, _nsl(BANK_F)],
                           in1=cst[l][:])
            eng.tensor_sub(out=fct[l][:], in0=fct[l][:],
                           in1=gt[l][:, _nsl(BANK_I)])
            eng.scalar_tensor_tensor(
                out=mt_[l][:], in0=gt[l][:, GSL], scalar=2.0,
                in1=gt[l][:, _nsl(BANK_I)],
                op0=mybir.AluOpType.mult, op1=mybir.AluOpType.mult,
            )
            eng.tensor_add(out=cst[l][:], in0=fct[l][:], in1=mt_[l][:])
            # tanh(c) = 2*sigmoid(2c) - 1
            nc.scalar.activation(out=tch[l][:], in_=cst[l][:],
                                 func=AFT.Sigmoid, scale=2.0)
            eng.tensor_scalar(
                out=tch[l][:], in0=tch[l][:], scalar1=2.0, scalar2=-1.0,
                op0=mybir.AluOpType.mult, op1=mybir.AluOpType.add,
            )
            h_new = st.tile([BL, D], F32, tag=f"hst{l}", bufs=2,
                            name=f"hst{l}n")
            eng.tensor_mul(out=h_new[:], in0=gt[l][:, OSL], in1=tch[l][:])
            hst[l] = h_new

        for rep in range(REPS):
          for t in range(T + 1):
            last = t == T
            first = t == 0
            gxt = None
            if not last:
                # stage this step's input-projection gates: SBUF->SBUF DMA
                # from the resident gx tile (no HBM traffic)
                gxt = wk.tile([BL, G], BF16, tag="gxt", bufs=3)
                nc.sync.dma_start(
                    out=gxt[:],
                    in_=gx_sb[
                        (t % TPM) * BL : (t % TPM + 1) * BL,
                        (t // TPM) * G : (t // TPM + 1) * G,
                    ],
                )

            # deferred lane-1 transpose: h1[t-2] -> hT[1]
            if t >= 2:
                transpose_h(pbl1, hst[1])
                hT1n = st.tile([128, KC * BL], BF16, tag="ht1", bufs=2,
                               name="hT1n")
                nc.vector.tensor_copy(out=hT1n[:], in_=pbl1[:, 0 : KC * BL])
                hT[1] = hT1n

            # lane-0 matmuls (step t): h0[t-1] @ W_hh0
            if not last:
                for n in range(NB):
                    mm_group(pbl0, n * 512, hT[0], whh0_sb, n, True, True)
            # lane-1 matmuls (step t-1): h0[t-1] @ W_ih1 + h1[t-2] @ W_hh1
            if not first:
                for n in range(NB):
                    mm_group(pbl1, n * 512, hT[0], wih1_sb, n, True, False)
                    mm_group(pbl1, n * 512, hT[1], whh1_sb, n, False, True)

            # lane-0 chain + transpose (priority: gates next iteration)
            if not last:
                nc.vector.tensor_add(out=gt[0][:, FI], in0=pbl0[:BL, FI],
                                     in1=gxt[:, FI])
                nc.vector.tensor_add(out=gt[0][:, GSL], in0=pbl0[:BL, GSL],
                                     in1=gxt[:, GSL])
                nc.vector.tensor_add(out=gt[0][:, OSL], in0=pbl0[:BL, OSL],
                                     in1=gxt[:, OSL])
                chain(0)
                transpose_h(pbl0, hst[0])
                hT0n = st.tile([128, KC * BL], BF16, tag="ht0", bufs=2,
                               name="hT0n")
                nc.vector.tensor_copy(out=hT0n[:], in_=pbl0[:, 0 : KC * BL])
                hT[0] = hT0n

            # lane-1 chain (h-transpose deferred to next iteration)
            if not first:
                nc.vector.tensor_add(out=gt[1][:, FI], in0=pbl1[:BL, FI],
                                     in1=bias1_sb[:, FI])
                nc.vector.tensor_add(out=gt[1][:, GSL], in0=pbl1[:BL, GSL],
                                     in1=bias1_sb[:, GSL])
                nc.vector.tensor_add(out=gt[1][:, OSL], in0=pbl1[:BL, OSL],
                                     in1=bias1_sb[:, OSL])
                chain(1)
                nc.sync.dma_start(out=out[:, t - 1, :], in_=hst[1][:])


_NC_CACHE = {}


def _get_nc():
    if "nc" not in _NC_CACHE:
        _NC_CACHE["nc"] = _build()
    return _NC_CACHE["nc"]


def _make_in_maps(inputs):
    import ml_dtypes

    bf16 = ml_dtypes.bfloat16

    tokens = np.asarray(inputs["prev_tgt_tokens"])[:, :T].astype(np.int32)  # [B, T]
    emb = np.ascontiguousarray(np.asarray(inputs["emb"], dtype=np.float32))
    W_ih = np.asarray(inputs["W_ih"], dtype=np.float32)
    W_hh = np.asarray(inputs["W_hh"], dtype=np.float32)
    b_ih = np.asarray(inputs["b_ih"], dtype=np.float32)
    b_hh = np.asarray(inputs["b_hh"], dtype=np.float32)
    hiddens = np.asarray(inputs["hiddens"], dtype=np.float32)
    cells = np.asarray(inputs["cells"], dtype=np.float32)

    def permute_gates(a, axis):
        # PyTorch gate order [i, f, g, o] -> kernel bank order [f, i, g, o]
        blocks = np.split(a, 4, axis=axis)
        return np.concatenate([blocks[1], blocks[0], blocks[2], blocks[3]], axis=axis)

    emb_bf = np.ascontiguousarray(emb.astype(bf16))
    wih0t = np.ascontiguousarray(permute_gates(W_ih[0].T, 1).astype(bf16))  # [D, G]
    whh0t = np.ascontiguousarray(permute_gates(W_hh[0].T, 1).astype(bf16))
    wih1t = np.ascontiguousarray(permute_gates(W_ih[1].T, 1).astype(bf16))
    whh1t = np.ascontiguousarray(permute_gates(W_hh[1].T, 1).astype(bf16))
    bias_all = permute_gates(b_ih + b_hh, 1)  # [2, G] fp32
    bias0 = np.ascontiguousarray(
        np.broadcast_to(bias_all[0][None, :], (128, G)).astype(bf16)
    )
    bias1 = np.ascontiguousarray(
        np.broadcast_to(bias_all[1][None, :], (BL, G)).astype(np.float32)
    )

    in_maps = []
    for core in range(N_CORES):
        sl = slice(core * BL, (core + 1) * BL)
        tok_tm = np.ascontiguousarray(tokens[sl].T.reshape(BL * T, 1))  # t-major
        ht = np.empty((2, 128, KC * BL), dtype=np.float32)
        for l in range(2):
            # [BL, D] -> h^T [D, BL] -> [KC, 128, BL] -> [128, KC, BL]
            htl = hiddens[l, sl].T.reshape(KC, 128, BL).transpose(1, 0, 2)
            ht[l] = htl.reshape(128, KC * BL)
        cin = np.ascontiguousarray(cells[:, sl, :])
        in_maps.append(
            {
                "tokens": tok_tm,
                "emb": emb_bf,
                "wih0t": wih0t,
                "whh0t": whh0t,
                "wih1t": wih1t,
                "whh1t": whh1t,
                "bias0": bias0,
                "bias1": bias1,
                "ht_init": np.ascontiguousarray(ht.astype(bf16)),
                "c_init": cin,
            }
        )
    return in_maps


def run(inputs, trace=False, **kwargs):
    """Build (cached), run on 8 cores, return (full_output, BassKernelResults)."""
    nc = _get_nc()
    in_maps = _make_in_maps(inputs)
    res = run_bass_kernel_spmd(
        nc, in_maps, core_ids=list(range(N_CORES)), trace=trace, **kwargs
    )
    out = np.concatenate([r["out"] for r in res.results], axis=0)  # [B, T, D]
    return out, res


def kernel(**inputs) -> np.ndarray:
    out, _ = run(inputs, trace=False)
    return out
